# revision 4
# baseline (speedup 1.0000x reference)
"""Trainium2 Bass kernel for the CRF loss — v4.

Time-segmented scan (see kernel2.py docstring).  v4 vs v3:
  - zdot rows ride the TT: the DVE multiply covers all 98 rows (x has
    ones in rows 96:98), so each SBUF state tile carries the previous
    state's zdot — no ACT PSUM copies at all.
  - all states live in one flat SBUF buffer per chain; zdots are DMA'd
    straight from its rows 96:98 in a few big chunks.
  - W=4 warmup, G=4 chains, PSUM tiles [98,256] x 3 bufs per chain.
"""

import os
import numpy as np
import ml_dtypes

import concourse.bass as bass
import concourse.tile as tile
from concourse import bacc, mybir
from concourse.bass_utils import run_bass_kernel_spmd

# ----------------------------------------------------------------------------
B, T, K = 512, 1024, 48
NCORES = 8

G = int(os.environ.get("CRF_G", "4"))        # chains (time-segments) per core
W = int(os.environ.get("CRF_W", "2"))        # warmup steps

NSEG = NCORES * G
SEG = T // NSEG
S = W + SEG                                  # real steps per chain
NSTEP = S + 1                                # steps incl. final zdot flush
COLS = 256
KP = 2 * K
KZ = KP + 2
OFF = float(np.log(K) + 0.5)


def _chunk_schedule(nstep):
    """Rounds per emission chunk: tiny first chunks so compute starts
    as soon as possible, then steady-state sizes."""
    sched = []
    rem = nstep
    for sz in (1, 2, 3):
        if rem <= 0:
            break
        take = min(sz, rem)
        sched.append(take)
        rem -= take
    while rem > 0:
        take = min(5, rem)
        sched.append(take)
        rem -= take
    return sched


CHUNKS = _chunk_schedule(NSTEP)
NZDMA = next(n for n in (5, 4, 3, 2, 1) if NSTEP % n == 0)  # zdot DMA windows
F32 = mybir.dt.float32
BF16 = mybir.dt.bfloat16
BF16_NP = ml_dtypes.bfloat16


def build_program():
    nc = bacc.Bacc(
        "TRN2", target_bir_lowering=False, debug=False,
        enable_asserts=False, num_devices=NCORES,
    )

    xbuf_d = nc.dram_tensor("xbuf", [KZ, NSTEP * G * COLS], BF16,
                            kind="ExternalInput").ap()
    wts_d = nc.dram_tensor("wts", [KP, KZ], BF16, kind="ExternalInput").ap()
    zbuf_d = nc.dram_tensor("zbuf", [2 * G, NSTEP * COLS], BF16,
                            kind="ExternalOutput").ap()

    zstride = (NSTEP // NZDMA) * COLS

    with tile.TileContext(nc) as tc:
        with (
            tc.tile_pool(name="const", bufs=1) as constp,
            tc.tile_pool(name="xch", bufs=3) as xpool,
            tc.tile_pool(name="ubig", bufs=1) as ubpool,
            tc.tile_pool(name="psum", bufs=2, space="PSUM") as pspool,
        ):
            wts = constp.tile([KP, KZ], BF16, tag="wts")
            nc.gpsimd.dma_start(wts[:], wts_d[:])

            u_init = constp.tile([KP, COLS], BF16, tag="uinit")
            nc.gpsimd.memset(u_init[:], 1.0)

            ub = [
                ubpool.tile([KZ, NSTEP * COLS], BF16, tag=f"ub{g}",
                            name=f"ub{g}")
                for g in range(G)
            ]
            ps = [None] * G

            r1 = 0
            for csz in CHUNKS:
                r0, r1 = r1, r1 + csz
                xch = xpool.tile([KZ, (r1 - r0) * G * COLS], BF16, tag="xch")
                nc.sync.dma_start(
                    xch[:], xbuf_d[:, r0 * G * COLS:r1 * G * COLS])
                for r in range(r0, r1):
                    for g in range(G):
                        ps[g] = pspool.tile([KZ, COLS], F32, tag=f"ps{g}",
                                            name=f"ps{g}")
                        rhs = (u_init[:] if r == 0
                               else ub[g][0:KP, (r - 1) * COLS:r * COLS])
                        nc.tensor.matmul(ps[g][:], wts[:], rhs)
                        nc.vector.tensor_tensor(
                            ub[g][:, r * COLS:(r + 1) * COLS],
                            ps[g][:],
                            xch[:, ((r - r0) * G + g) * COLS:
                                ((r - r0) * G + g + 1) * COLS],
                            mybir.AluOpType.mult,
                        )
                        # drain zdot rows once a window of steps is final
                        if (r + 1) % (NSTEP // NZDMA) == 0:
                            zi = (r + 1) // (NSTEP // NZDMA) - 1
                            nc.scalar.dma_start(
                                zbuf_d[2 * g:2 * g + 2,
                                       zi * zstride:(zi + 1) * zstride],
                                ub[g][KP:KZ, zi * zstride:(zi + 1) * zstride],
                            )

    nc.compile()
    return nc


_PROG_CACHE = {}
LAST_RESULTS = None


def _get_program():
    if "p" not in _PROG_CACHE:
        _PROG_CACHE["p"] = build_program()
    return _PROG_CACHE["p"]


# ----------------------------------------------------------------------------
# Host side


def _quantized_params(prior, transition, final_transition):
    M2 = np.exp(np.asarray(transition, np.float64))
    M2q = M2.astype(BF16_NP).astype(np.float64)
    expFq = np.exp(np.asarray(final_transition, np.float64)
                   ).astype(BF16_NP).astype(np.float64)
    wts = np.zeros((KP, KZ), np.float32)
    wts[0:K, 0:K] = M2q.T
    wts[K:KP, K:KP] = M2q.T
    wts[0:K, KP] = expFq
    wts[K:KP, KP + 1] = expFq
    return wts.astype(BF16_NP), M2q, expFq


def _synthetic_warmup(M2q, prior):
    p = np.exp(np.asarray(prior, np.float64))
    v = np.linalg.solve(M2q, p)
    u = np.ones(K, np.float64)
    for _ in range(W - 1):
        u = (M2q @ u) / K
    return v / (M2q @ u)


def _host_inputs(emission_scores, prior, transition, final_transition):
    wts, M2q, expFq = _quantized_params(prior, transition, final_transition)
    x_syn = _synthetic_warmup(M2q, prior)
    X = np.exp(emission_scores.astype(np.float32) - OFF).astype(BF16_NP)

    in_maps = []
    for j in range(NCORES):
        xb = np.empty((KZ, NSTEP, G, COLS), BF16_NP)
        xb[KP:KZ] = BF16_NP(1.0)             # zdot passthrough rows
        xb[0:KP, S:] = BF16_NP(1.0)          # flush/pad steps
        for g in range(G):
            seg = G * j + g
            t0 = seg * SEG
            tvec = t0 - W + np.arange(S)
            tsafe = np.clip(tvec, 0, T - 1)
            arr = X[:, tsafe, :]
            arr = arr.reshape(2, COLS, S, K)
            arr = np.transpose(arr, (0, 3, 2, 1))      # [bb, k, i, col]
            xb[0:KP, :S, g, :] = arr.reshape(KP, S, COLS)
            if seg == 0:
                xb[0:KP, 0:W - 1, g, :] = BF16_NP(1.0 / K)
                xs = np.tile(x_syn, 2).astype(BF16_NP)
                xb[0:KP, W - 1, g, :] = xs[:, None]
        in_maps.append({
            "xbuf": np.ascontiguousarray(xb.reshape(KZ, NSTEP * G * COLS)),
            "wts": wts,
        })
    return in_maps, M2q, expFq


def _host_path(emission_scores, lengths, tags, prior, transition,
               final_transition):
    b_idx = np.arange(B)
    emis = emission_scores.astype(np.float64)
    emis_tag = np.take_along_axis(emis, tags[:, :, None], axis=2)[..., 0]
    valid = np.arange(T)[None, :] < lengths[:, None]
    pr = np.asarray(prior, np.float64)[tags[:, 0]]
    tr = np.asarray(transition, np.float64)[tags[:, 1:], tags[:, :-1]]
    valid_tr = (np.arange(1, T)[None, :] < lengths[:, None])
    fin = np.asarray(final_transition, np.float64)[tags[b_idx, lengths - 1]]
    return (pr + np.where(valid_tr, tr, 0.0).sum(axis=1)
            + np.where(valid, emis_tag, 0.0).sum(axis=1) + fin)


def _finalize(results, lengths, path):
    Ls = np.zeros((NSEG, S, B), np.float64)
    for j in range(NCORES):
        zb = np.asarray(results[j]["zbuf"], np.float64)
        for g in range(G):
            s = G * j + g
            zrows = zb[2 * g:2 * g + 2]
            m = np.arange(1, S + 1)
            cols = (m[:, None] * COLS + np.arange(COLS)[None, :])
            z = np.concatenate([zrows[0][cols], zrows[1][cols]], axis=1)
            logz = np.log(np.maximum(np.abs(z), 1e-300))
            if s == 0:
                n = np.maximum(0, np.arange(S) - (W - 1))
            else:
                n = np.arange(S) + 1
            Ls[s] = logz + OFF * n[:, None]

    logc = np.zeros((NSEG, B), np.float64)
    for s in range(1, NSEG):
        logc[s] = Ls[s - 1, W + SEG - 1] + logc[s - 1] - Ls[s, W - 1]
    t_star = lengths - 1
    s_star = t_star // SEG
    i_star = W + (t_star % SEG)
    logZ = Ls[s_star, i_star, np.arange(B)] + logc[s_star, np.arange(B)]
    return np.float32(np.mean(logZ - path))


def kernel(emission_scores, lengths, tags, prior, transition, final_transition):
    emission_scores = np.asarray(emission_scores, np.float32)
    lengths = np.clip(np.asarray(lengths), 1, T).astype(np.int64)
    tags = np.asarray(tags).astype(np.int64)

    nc = _get_program()
    in_maps, _, _ = _host_inputs(emission_scores, prior, transition,
                                 final_transition)

    trace = os.environ.get("CRF_TRACE", "0") == "1"
    res = run_bass_kernel_spmd(nc, in_maps, list(range(NCORES)), trace=trace)
    global LAST_RESULTS
    LAST_RESULTS = res

    path = _host_path(emission_scores, lengths, tags, prior, transition,
                      final_transition)
    return _finalize(res.results, lengths, path)


if __name__ == "__main__":
    rng = np.random.default_rng(0)
    inputs = {
        "emission_scores": rng.standard_normal((B, T, K), dtype=np.float32),
        "lengths": rng.integers(1, T + 1, size=(B,)).astype(np.int64),
        "tags": rng.integers(0, K, size=(B, T)).astype(np.int64),
        "prior": (0.1 * rng.standard_normal(K)).astype(np.float32),
        "transition": (0.1 * rng.standard_normal((K, K))).astype(np.float32),
        "final_transition": (0.1 * rng.standard_normal(K)).astype(np.float32),
    }
    print("loss =", kernel(**inputs))
